# revision 44
# baseline (speedup 1.0000x reference)
"""Trainium2 Bass kernel for a 2-layer DGCN (graph conv) on 8 NeuronCores.

Reference computation (fp32):
    h1  = relu(IFadj @ (x @ W1) + b1)         # [N, NHID]
    out = BN(adj @ (h1 @ W2) + b2)            # [N, OUTD], BN in eval mode

Distribution: rows of IFadj / adj are sharded across 8 cores
(row-parallel graph partitioning); x and the weights are replicated.
Per core (rows R_k), v12 schedule:

  phase S: S = x @ W1 computed IN FULL on every core, fp8e4 DoubleRow
           (one matmul per adjacent feature-block pair), W1 pre-scaled
           by 16 on the host so its entries clear the e4m3 subnormal
           range, the 1/16 folded into the PSUM eviction scale. Full
           replication costs ~148k PE cycles -- cheaper than the
           S-AllGather critical path (cross-core launch-skew barrier
           ~45-55us + two ~25us gathers) it replaces, and it removes
           every S collective, staging DMA, and redundancy schedule.
  phase C: h1T = relu(S^T @ B + bias), two i-half passes, fp8e4
           DoubleRow over adjacent m-tile pairs. After each pass:
           z-half = h1 @ (W2/4) in fp8, Z-AllGather chunk fired.
           The first Z-AllGather absorbs the launch-skew barrier,
           which completes long before the trigger.
  phase D: outT = Z-as-lhsT vs adjT_k rhs, fp8e4 DoubleRow matmuls;
           BN fused on the PSUM evict ((ob, ih)-split stops so the
           final evictions overlap the last matmuls).

Engine/queue discipline (hard-won): every dma_start dispatches
serially on its issuing engine's queue, and the tile scheduler may
reorder same-engine dispatches, so a collective-gated DMA can
head-block urgent loads behind it. Therefore: SP (nc.sync) carries
setup + the gather-gated z staging (everything behind it is needed
even later); Activation (nc.scalar) carries the big streaming loads
(IFadj dual tiles, adj pair tiles), never collective-gated; GpSimd
carries only the collective triggers. PSUM Copy-evictions run on the
Vector engine. All transfers are plain 2D/3D slices: the host
pre-permutes x/W1/W2 partition-major and IFadj/adj pair-interleaved;
the z bounce is written p-major so gathered z is plain-sliceable.

Precision: x, W1 (x16), S, IFadj, adj, z (W2/4-scaled) all run in
fp8e4; W2 matmul in bf16. IFadj is CENTERED: B = IFadj - 1/2 maps
U[0,1] into [-1/2,1/2], and the exact mean term 1/2*colsum(S)_j
(colsum from an exact host matvec) is folded into the layer-1 bias,
so the dominant mean flow carries no fp8 noise. The 1/4 z-fold is
undone in the BN scale. Measured end-to-end rel err ~2.2e-3 vs the
2e-2 gate.
"""

import numpy as np
import ml_dtypes

NCORES = 8
N = 8192
NFEAT = 1024
NHID = 512
OUTD = 256
ROWS = N // NCORES  # 1024
P = 128
BN_EPS = 1e-5

CB = NFEAT // P   # 8  c-blocks (x feature contraction)
IB = ROWS // P    # 8  i-blocks per node block
JB = NHID // P    # 4  j-blocks (hidden)
MT = N // P       # 64 m-tiles (global node contraction)
HF = 512          # matmul moving free dim (PSUM bank limit)
IH = ROWS // HF   # 2 i-halves of the local row range
OB = OUTD // P    # 2 output-feature blocks
GC = 2            # z allgather chunks
QT = 4            # m-tiles per (chunk, core-block) quarter

_BF16 = ml_dtypes.bfloat16
_F8 = ml_dtypes.float8_e4m3

_cache = {}


def _build():
    import concourse.mybir as mybir
    import concourse.tile as tile
    from concourse import bacc

    dt = mybir.dt
    f32 = dt.float32
    bf16 = dt.bfloat16
    f8 = dt.float8e4
    DR = mybir.MatmulPerfMode.DoubleRow
    MULT = mybir.AluOpType.mult

    nc = bacc.Bacc("TRN2", target_bir_lowering=False, debug=False,
                   num_devices=NCORES)

    # full x^T in fp8, block-major partition-major: slab g holds node
    # block g as [P, CB, ROWS] (pair-sliceable along CB)
    xG_e = nc.dram_tensor("xG", [P, NCORES * CB * ROWS], f8,
                          kind="ExternalInput")
    # 16*W1 in fp8, partition-major [P, CB*NHID]
    w1_e = nc.dram_tensor("w1", [P, CB * NHID], f8, kind="ExternalInput")
    # centered IFadj^T in fp8, pair-interleaved and split by column
    # half on the host: row ih*4096 + pair*P + p, col t*HF + c holds
    # IFadjT[(2*pair+t)*P + p, ih*HF + c] - 1/2
    ifadjH_e = nc.dram_tensor("ifadjH", [IH * N // 2, 2 * HF], f8,
                              kind="ExternalInput")
    # adj rows pair-interleaved on host: row pair*P+p holds m-tiles
    # (2*pair, 2*pair+1) side by side -> [P, 2, ROWS] is a plain slice
    adjP_e = nc.dram_tensor("adjP", [N // 2, 2 * ROWS], f8,
                            kind="ExternalInput")
    # W2/4, partition-major: [P, JB*OUTD]
    w2_e = nc.dram_tensor("w2", [P, JB * OUTD], bf16, kind="ExternalInput")
    # layer-1 bias + 1/2*colsum(S) fold, [P, JB]
    b1p_e = nc.dram_tensor("b1p", [P, JB], f32, kind="ExternalInput")
    bnsc_e = nc.dram_tensor("bnsc", [P, OB], f32, kind="ExternalInput")
    bnbi_e = nc.dram_tensor("bnbi", [P, OB], f32, kind="ExternalInput")
    # outT: [OUTD, ROWS]; the host transposes each core's block.
    out_e = nc.dram_tensor("out", [OUTD, ROWS], f32, kind="ExternalOutput")

    groups = [list(range(NCORES))]

    def allgather(g_in, g_out):
        nc.gpsimd.collective_compute(
            "AllGather", mybir.AluOpType.bypass, replica_groups=groups,
            ins=[g_in[:]], outs=[g_out[:]])

    with tile.TileContext(nc) as tc:
        with (
            tc.tile_pool(name="const", bufs=1) as const,
            tc.tile_pool(name="xslab", bufs=3) as xslab_p,
            tc.tile_pool(name="sfull", bufs=1) as sfull_p,
            tc.tile_pool(name="h1", bufs=1) as h1_p,
            tc.tile_pool(name="zsb", bufs=1) as z_p,
            tc.tile_pool(name="zchunk", bufs=8) as zchunk_p,
            tc.tile_pool(name="astream", bufs=8) as astream,
            tc.tile_pool(name="apair", bufs=4) as apair_p,
            tc.tile_pool(name="outsb", bufs=1) as outsb_p,
            tc.tile_pool(name="dram", bufs=1, space="DRAM") as dram,
        ):
            # z bounce is p-major: row p holds (t, o), t = chunk-local
            # i-block -> gathered z is plain-sliceable per core block
            z_bounce = [dram.tile([P, QT * OUTD], f8, name=f"zb{c}")
                        for c in range(GC)]
            z_all = [dram.tile([P * NCORES, QT * OUTD], f8,
                               addr_space="Shared", name=f"za{c}")
                     for c in range(GC)]

            # w1 + the first x slab lead the SP queue so the first
            # matmul fires as early as possible
            w1_sb = const.tile([P, CB, NHID], f8)
            nc.sync.dma_start(w1_sb[:], w1_e[:])

            # tiny dummy collectives keep the ncfw/CC path warm between
            # the start-of-kernel barrier and the first real gather
            dmy_sb = const.tile([P, 4], f32)
            nc.gpsimd.memset(dmy_sb[:], 0.0)
            dmy_in = dram.tile([P, 4], f32, name="dmi0")
            dmy_out = dram.tile([P * NCORES, 4], f32, addr_space="Shared",
                                name="dmo0")
            nc.sync.dma_start(dmy_in[:], dmy_sb[:])
            allgather(dmy_in, dmy_out)

            b1p_sb = const.tile([P, JB], f32)
            nc.sync.dma_start(b1p_sb[:], b1p_e[:])
            w2_sb = const.tile([P, JB, OUTD], bf16)
            nc.sync.dma_start(w2_sb[:], w2_e[:])
            bnsc_sb = const.tile([P, OB], f32)
            nc.sync.dma_start(bnsc_sb[:], bnsc_e[:])
            bnbi_sb = const.tile([P, OB], f32)
            nc.sync.dma_start(bnbi_sb[:], bnbi_e[:])

            s_full = sfull_p.tile([P, MT, NHID], f8)

            # ---- phase S: S = x @ (16*W1) / 16 for ALL node blocks,
            # fp8 DoubleRow over feature-block pairs, 4-bank sub-passes
            # so evictions overlap the next sub-pass's matmuls.
            with tc.tile_pool(name="psA", bufs=1, space="PSUM") as psA:
                for g in range(NCORES):
                    xs = xslab_p.tile([P, CB, ROWS], f8, tag="xslab")
                    nc.sync.dma_start(
                        xs[:], xG_e[:, g * CB * ROWS:(g + 1) * CB * ROWS])
                    for hh in range(2):
                        ps_s = [psA.tile([P, NHID], f32,
                                         name=f"ps{g}_{hh}_{t}",
                                         tag=f"pa{hh * 4 + t}")
                                for t in range(4)]
                        for cb in range(0, CB, 2):
                            for t in range(4):
                                ib = hh * 4 + t
                                nc.tensor.matmul(
                                    ps_s[t][:],
                                    xs[:, cb:cb + 2,
                                       ib * P:(ib + 1) * P],
                                    w1_sb[:, cb:cb + 2, :],
                                    start=(cb == 0), stop=(cb == CB - 2),
                                    perf_mode=DR)
                        for t in range(4):
                            nc.vector.tensor_scalar(
                                s_full[:, g * IB + hh * 4 + t, :],
                                ps_s[t][:], 0.0625, None, MULT)

            # second warmer, gated on the end of phase S (~90us in)
            dmy2_in = dram.tile([P, 4], f8, name="dmi2")
            dmy2_out = dram.tile([P * NCORES, 4], f8, addr_space="Shared",
                                 name="dmo2")
            nc.sync.dma_start(dmy2_in[:], s_full[:, MT - 1, 0:4])
            allgather(dmy2_in, dmy2_out)

            h1T = h1_p.tile([P, JB, ROWS], bf16)
            z_sb = z_p.tile([P, IB, OUTD], f8)

            # ---- phase C, i-half pass ih: h1T half accumulated over
            # all 64 m-tiles as DoubleRow pairs; then relu-evict, z-half
            # in fp8, Z-AllGather chunk.
            def l1_pass(ih, psh, psz):
                psum_h = [psh.tile([P, HF], f32, name=f"ph{jb}_{ih}",
                                   tag=f"ph{jb}")
                          for jb in range(JB)]
                for pj in range(MT // 2):
                    a_dual = astream.tile([P, 2, HF], f8, tag="adual")
                    nc.scalar.dma_start(
                        a_dual[:],
                        ifadjH_e[ih * (N // 2) + pj * P:
                                 ih * (N // 2) + (pj + 1) * P, :])
                    for jb in range(JB):
                        nc.tensor.matmul(
                            psum_h[jb][:],
                            s_full[:, 2 * pj:2 * pj + 2,
                                   jb * P:(jb + 1) * P],
                            a_dual[:],
                            start=(pj == 0),
                            stop=(pj == MT // 2 - 1),
                            perf_mode=DR,
                        )
                # epilogue: relu(psum + bias) evicted on the Vector
                # engine per jb, with the z partial matmuls for that jb
                # pipelined right behind (4 i-block accumulators)
                zps = [psz.tile([P, OUTD], f32, name=f"zp{ih}_{t}",
                                tag=f"z{t}")
                       for t in range(IB // IH)]
                for jb in range(JB):
                    nc.vector.tensor_scalar(
                        h1T[:, jb, ih * HF:(ih + 1) * HF],
                        psum_h[jb][:], b1p_sb[:, jb:jb + 1], 0.0,
                        mybir.AluOpType.add, mybir.AluOpType.max)
                    for t in range(IB // IH):
                        ib = ih * (IB // IH) + t
                        nc.tensor.matmul(
                            zps[t][:],
                            h1T[:, jb, ib * P:(ib + 1) * P],
                            w2_sb[:, jb, :],
                            start=(jb == 0), stop=(jb == JB - 1),
                        )
                for t in range(IB // IH):
                    ib = ih * (IB // IH) + t
                    nc.vector.tensor_scalar(
                        z_sb[:, ib, :], zps[t][:], 1.0, None, MULT)
                    nc.sync.dma_start(
                        z_bounce[ih][:, t * OUTD:(t + 1) * OUTD],
                        z_sb[:, ib, :])
                allgather(z_bounce[ih], z_all[ih])

            with (
                tc.tile_pool(name="psh", bufs=1, space="PSUM") as psh,
                tc.tile_pool(name="psz", bufs=1, space="PSUM") as psz,
            ):
                for ih in range(IH):
                    l1_pass(ih, psh, psz)

            # ---- phase D: outT[o, i] = sum_m Z[m, o] * adjT[m, i]
            # fp8 DoubleRow, one matmul per adjacent m-tile pair.
            # z_all[c] row k*P+p holds (t, o) = z[k*ROWS + c*RPC + t*P+p]
            # -> m-tile of (c, k, t) is 8k + 4c + t.
            outT_sb = outsb_p.tile([P, OB, ROWS], f32)
            with tc.tile_pool(name="ps4", bufs=1, space="PSUM") as ps4:
                psum_o = [[ps4.tile([P, HF], f32, name=f"po{ob}_{ih}",
                                    tag=f"po{ob}_{ih}")
                           for ih in range(IH)] for ob in range(OB)]
                first = True
                for c in range(GC):
                    for k in range(NCORES):
                        zc_sb = zchunk_p.tile([P, QT, OUTD], f8,
                                              tag="zchunk")
                        nc.sync.dma_start(
                            zc_sb[:], z_all[c][k * P:(k + 1) * P, :])
                        last_grp = (c == GC - 1 and k == NCORES - 1)
                        a_prs = {}
                        for pr in range(0, QT, 2):
                            mt = IB * k + QT * c + pr
                            a_pair = apair_p.tile([P, 2, ROWS], f8,
                                                  tag="apair")
                            nc.scalar.dma_start(
                                a_pair[:],
                                adjP_e[(mt // 2) * P:(mt // 2 + 1) * P, :])
                            a_prs[pr] = a_pair
                        # last group runs (ob, ih)-outer so each output
                        # quarter stops as early as possible and its BN
                        # eviction + store overlap the remaining matmuls
                        if last_grp:
                            for ob in range(OB):
                                for ih in range(IH):
                                    for pr in range(0, QT, 2):
                                        nc.tensor.matmul(
                                            psum_o[ob][ih][:],
                                            zc_sb[:, pr:pr + 2,
                                                  ob * P:(ob + 1) * P],
                                            a_prs[pr][:, :,
                                                      ih * HF:
                                                      (ih + 1) * HF],
                                            start=False,
                                            stop=(pr == QT - 2),
                                            perf_mode=DR,
                                        )
                        else:
                            for pr in range(0, QT, 2):
                                for ob in range(OB):
                                    for ih in range(IH):
                                        nc.tensor.matmul(
                                            psum_o[ob][ih][:],
                                            zc_sb[:, pr:pr + 2,
                                                  ob * P:(ob + 1) * P],
                                            a_prs[pr][:, :,
                                                      ih * HF:
                                                      (ih + 1) * HF],
                                            start=first, stop=False,
                                            perf_mode=DR,
                                        )
                                first = False
                # fused BN affine on PSUM evict: out = psum*scale + bias;
                # store per quarter so the tail pipelines
                for ob in range(OB):
                    for ih in range(IH):
                        nc.vector.tensor_scalar(
                            outT_sb[:, ob, ih * HF:(ih + 1) * HF],
                            psum_o[ob][ih][:],
                            bnsc_sb[:, ob:ob + 1],
                            bnbi_sb[:, ob:ob + 1],
                            mybir.AluOpType.mult,
                            mybir.AluOpType.add)
                        nc.sync.dma_start(
                            out_e[ob * P:(ob + 1) * P,
                                  ih * HF:(ih + 1) * HF],
                            outT_sb[:, ob, ih * HF:(ih + 1) * HF])

    nc.compile()
    return nc


def _get_nc():
    if "nc" not in _cache:
        _cache["nc"] = _build()
    return _cache["nc"]


def kernel(x, IFadj, adj, W1, b1, W2, b2, bn_gamma, bn_beta, bn_mean, bn_var):
    from concourse.bass_utils import run_bass_kernel_spmd

    x = np.asarray(x, dtype=np.float32)
    IFadj = np.asarray(IFadj, dtype=np.float32)
    adj = np.asarray(adj, dtype=np.float32)
    W1 = np.asarray(W1, dtype=np.float32)
    b1 = np.asarray(b1, dtype=np.float32)
    W2 = np.asarray(W2, dtype=np.float32)
    b2 = np.asarray(b2, dtype=np.float32)
    bn_gamma = np.asarray(bn_gamma, dtype=np.float32)
    bn_beta = np.asarray(bn_beta, dtype=np.float32)
    bn_mean = np.asarray(bn_mean, dtype=np.float32)
    bn_var = np.asarray(bn_var, dtype=np.float32)

    # host-side prep: layout permutes + casts only (plus two trivial
    # folds: W1 x16 for fp8 range, W2/4 for fp8 z range, and the exact
    # colsum(S) mean term from one host matvec).
    w1b = np.ascontiguousarray(
        (16.0 * W1).astype(_F8).reshape(CB, P, NHID).transpose(1, 0, 2)
        .reshape(P, CB * NHID))
    w2b = np.ascontiguousarray(
        (W2 * 0.25).astype(_BF16).reshape(JB, P, OUTD)
        .transpose(1, 0, 2).reshape(P, JB * OUTD))
    colsum = x.sum(axis=0, dtype=np.float64).astype(np.float32) @ W1
    b1c = b1 + 0.5 * colsum
    b1p = np.ascontiguousarray(b1c.reshape(JB, P).T)  # [P, JB]
    inv = bn_gamma / np.sqrt(bn_var + BN_EPS)
    bias_tot = b2 * inv + bn_beta - bn_mean * inv
    bnsc = np.ascontiguousarray((4.0 * inv).reshape(OB, P).T)   # [P, OB]
    bnbi = np.ascontiguousarray(bias_tot.reshape(OB, P).T)      # [P, OB]

    # full x^T fp8, block-major partition-major (replicated)
    x8 = x.astype(_F8)
    xG = np.ascontiguousarray(
        np.stack([
            x8[g * ROWS:(g + 1) * ROWS].T
            .reshape(CB, P, ROWS).transpose(1, 0, 2)
            for g in range(NCORES)], axis=1)
        .reshape(P, NCORES * CB * ROWS))

    in_maps = []
    for k in range(NCORES):
        r0, r1 = k * ROWS, (k + 1) * ROWS
        # centered IFadj^T in fp8: [m, col] -> [ih, pair, p, t, c]
        A8 = (IFadj[r0:r1].T - np.float32(0.5)).astype(_F8)  # [N, ROWS]
        ifadjH = np.ascontiguousarray(
            A8.reshape(N // 256, 2, P, IH, HF).transpose(3, 0, 2, 1, 4)
            .reshape(IH * N // 2, 2 * HF))
        adjT8 = np.ascontiguousarray(adj[r0:r1].T).astype(_F8)  # [N, ROWS]
        # pair-interleave: row pair*P+p = m-tiles (2p, 2p+1) side by side
        adjP = np.ascontiguousarray(
            adjT8.reshape(N // 256, 2, P, ROWS).transpose(0, 2, 1, 3)
            .reshape(N // 2, 2 * ROWS))
        in_maps.append({
            "xG": xG,
            "ifadjH": ifadjH,
            "adjP": adjP,
            "w1": w1b,
            "w2": w2b,
            "b1p": b1p,
            "bnsc": bnsc,
            "bnbi": bnbi,
        })

    global _last_in_maps
    _last_in_maps = in_maps

    nc = _get_nc()
    try:
        res = run_bass_kernel_spmd(nc, in_maps, list(range(NCORES)))
    except Exception:
        # transient device wedge (NRT_EXEC_UNIT_UNRECOVERABLE etc.) --
        # a straight retry has been observed to recover
        import time
        time.sleep(2.0)
        res = run_bass_kernel_spmd(nc, in_maps, list(range(NCORES)))
    # per-core output is outT [OUTD, ROWS]; transpose back and stack rows
    return np.concatenate(
        [np.ascontiguousarray(res.results[k]["out"].T)
         for k in range(NCORES)], axis=0)
